# revision 23
# baseline (speedup 1.0000x reference)
"""Trainium2 Bass kernel: batch-parallel tanh-projected attention.

Reference (per batch element, 8 elements total):
    qh = tanh(q @ Wq + bq); kh = tanh(k @ Wk + bk); vh = tanh(v @ Wv + bv)
    out = softmax(qh @ kh^T, axis=-1) @ vh

Sharding: data-parallel over batch B=8 across the 8 NeuronCores; the small
256x32 projection weights are replicated.

Per-core algorithm (all in "transposed" layouts so the 2048x2048 attention
matrix never needs transposing):
  - q/k/v cast f32->bf16 during DMA (SWDGE), transposed to [DIN, n] layout
    with the DMA xbar transpose engine (keeps TensorE/DVE free).
  - Projections produce hT4 = [128, 2048]: partition 32*i + c holds channel
    c of qh^T/kh^T, replicated 4x (replicated weight columns) -> enables
    4-way TensorE row-group packing for the K=32 score matmuls.
  - S^T = kh @ qh^T per key-tile ([128 keys, 512 q] PSUM); exp without
    max-subtraction (|S| <= 32 guaranteed by tanh; measured ~13). Most exp
    tiles run on ScalarE (exact); a subset runs on DVE via a Schraudolph
    bit-trick exp in bf16 space (int16(S*a + b) bit-pattern == bf16 exp
    approximation, one tensor_scalar op) to balance engine load.
  - O^T accumulated as [vh | 1]^T @ exp(S^T): the ones column yields the
    softmax denominator for free. 2-way column-group packing (M=33 -> 64-
    aligned column strips).
  - PE-transpose O^T chunks, divide by denominator, single output DMA.
"""

import numpy as np

B, N, M, DIN, DH = 8, 2048, 2048, 256, 32
P = 128
NT = N // P  # 16 row tiles
QC = 512  # q-chunk (matmul moving dim)
NQC = N // QC  # 4

# Schraudolph bf16-space exp: bitcast(int16(x * 128*log2(e) + (127*128 - C)))
EXP_A = float(128.0 / np.log(2.0))
EXP_B = float(127.0 * 128.0 - 5.25)
DVE_ROUNDS = (1, 3, 5, 7)  # rounds (of 8 per q-chunk) whose exp runs on DVE


def _build():
    import concourse.mybir as mybir
    import concourse.tile as tile
    from concourse import bacc
    from concourse.masks import make_identity

    fp32 = mybir.dt.float32
    bf16 = mybir.dt.bfloat16
    i16 = mybir.dt.int16

    nc = bacc.Bacc("TRN2", target_bir_lowering=False, debug=False)

    q_d = nc.dram_tensor("q", [N, DIN], fp32, kind="ExternalInput")
    k_d = nc.dram_tensor("k", [M, DIN], fp32, kind="ExternalInput")
    v_d = nc.dram_tensor("v", [M, DIN], fp32, kind="ExternalInput")
    wq_d = nc.dram_tensor("Wq", [DIN, DH], fp32, kind="ExternalInput")
    wk_d = nc.dram_tensor("Wk", [DIN, DH], fp32, kind="ExternalInput")
    wv_d = nc.dram_tensor("Wv", [DIN, DH], fp32, kind="ExternalInput")
    bq_d = nc.dram_tensor("bq", [DH], fp32, kind="ExternalInput")
    bk_d = nc.dram_tensor("bk", [DH], fp32, kind="ExternalInput")
    bv_d = nc.dram_tensor("bv", [DH], fp32, kind="ExternalInput")
    out_d = nc.dram_tensor("out", [N, DH], fp32, kind="ExternalOutput")

    xdram = {"q": q_d, "k": k_d, "v": v_d}
    wdram = {"q": wq_d, "k": wk_d, "v": wv_d}
    bdram = {"q": bq_d, "k": bk_d, "v": bv_d}

    with tile.TileContext(nc) as tc:
        with (
            tc.tile_pool(name="const", bufs=1) as const,
            tc.tile_pool(name="stage", bufs=1) as stage,
            tc.tile_pool(name="sb", bufs=1) as sb,
            tc.tile_pool(name="expp", bufs=4) as expp,
            tc.tile_pool(name="osb", bufs=2) as osb,
            tc.tile_pool(name="pbig", bufs=3, space="PSUM") as pbig,
            tc.tile_pool(name="po", bufs=1, space="PSUM") as po,
            tc.tile_pool(name="pt2", bufs=1, space="PSUM") as pt2,
        ):
            # ---- constants ----
            id_bf = const.tile([P, P], bf16)
            make_identity(nc, id_bf[:])
            id_f32 = const.tile([P, P], fp32)
            make_identity(nc, id_f32[:])

            w4 = {}
            bias = {}
            for name in ("q", "k", "v"):
                wf = const.tile([P, 2, DH], fp32, tag=f"wf_{name}", name=f"wf_{name}")
                nc.sync.dma_start(
                    wf[:], wdram[name][:].rearrange("(o p) c -> p o c", p=P)
                )
                w4t = const.tile(
                    [P, 2, 4 * DH], bf16, tag=f"w4_{name}", name=f"w4_{name}"
                )
                for j in range(4):
                    nc.vector.tensor_copy(w4t[:, :, j * DH : (j + 1) * DH], wf[:])
                w4[name] = w4t

                bt = const.tile([P, 1], fp32, tag=f"b_{name}", name=f"b_{name}")
                for i in range(4):
                    nc.sync.dma_start(
                        bt[i * DH : (i + 1) * DH, :],
                        bdram[name][:].rearrange("(c one) -> c one", one=1),
                    )
                bias[name] = bt

            xT = {}
            hT4 = {}
            for name in ("q", "k", "v"):
                xT[name] = sb.tile([P, 2, N], bf16, tag=f"xT_{name}", name=f"xT_{name}")
                hT4[name] = sb.tile([P, N], bf16, tag=f"hT4_{name}", name=f"hT4_{name}")

            # Input path: SWDGE cast-DMA chunks (f32->bf16), PE-transpose
            # 128x128 tiles, copy PSUM->SBUF alternating DVE/ScalarE (splits
            # the copyback load across both engines; ScalarE is idle during
            # the input phase since exp hasn't started).
            def load_and_transpose(name, g):
                src = xdram[name][:].rearrange("(t p) d -> p t d", p=P)
                xbf = stage.tile(
                    [P, 4, DIN], bf16, tag=f"xbf_{name}_{g}",
                    name=f"xbf_{name}_{g}",
                )
                nc.gpsimd.dma_start(xbf[:], src[:, 4 * g : 4 * g + 4, :])
                for o in range(2):
                    ptp = pbig.tile([P, 4, P], bf16, tag="big")
                    for i in range(4):
                        nc.tensor.transpose(
                            ptp[:, i, :],
                            xbf[:, i, o * P : (o + 1) * P],
                            id_bf[:],
                        )
                    dst = xT[name][:, o, 512 * g : 512 * (g + 1)]
                    if (g + o) % 2 == 0:
                        nc.vector.tensor_copy(dst, ptp[:])
                    else:
                        nc.scalar.copy(dst, ptp[:])

            def project(name, ch):
                # hT4 = tanh(W4^T @ xT + b), bf16, 4x-replicated channels
                ph = pbig.tile([P, 2, QC], fp32, tag="big")
                for nh in range(2):
                    for o in range(2):
                        nc.tensor.matmul(
                            ph[:, nh, :],
                            w4[name][:, o, :],
                            xT[name][
                                :, o, 1024 * ch + 512 * nh : 1024 * ch + 512 * (nh + 1)
                            ],
                            start=(o == 0),
                            stop=(o == 1),
                        )
                nc.scalar.activation(
                    hT4[name][:, 1024 * ch : 1024 * (ch + 1)].rearrange(
                        "p (a b) -> p a b", a=2
                    ),
                    ph[:],
                    mybir.ActivationFunctionType.Tanh,
                    bias=bias[name][:],
                )

            # vh_aug: [P, NT, DH+1] bf16 (row-major vh tiles + ones col)
            vh_aug = sb.tile([P, NT, DH + 1], bf16)
            nc.gpsimd.memset(vh_aug[:, :, DH : DH + 1], 1.0)

            def vh_aug_fill(g):
                pv = pbig.tile([P, 4, DH], bf16, tag="big")
                for i in range(4):
                    kt = 4 * g + i
                    nc.tensor.transpose(
                        pv[:, i, :],
                        hT4["v"][0:DH, P * kt : P * (kt + 1)],
                        id_bf[0:DH, 0:DH],
                    )
                nc.vector.tensor_copy(vh_aug[:, 4 * g : 4 * g + 4, 0:DH], pv[:])

            # Progressive setup: the main loop's round r of any chunk needs
            # only key/value tiles 2r,2r+1 and q-chunk c — so stage the
            # first halves of k/q/v (casts g0,g1 -> proj ch0 -> vh_aug
            # tiles 0-7), then EMIT chunk 0's rounds 0-3 before the second
            # setup half, so main-loop work (and its PSUM slot requests)
            # interleaves with the remaining input processing.
            def setup_half(h):
                for g in (2 * h, 2 * h + 1):
                    for name in ("k", "q", "v"):
                        load_and_transpose(name, g)
                for name in ("k", "q", "v"):
                    project(name, h)
                vh_aug_fill(2 * h)
                vh_aug_fill(2 * h + 1)

            # ---- main attention loop ----
            out_sb = sb.tile([P, NT, DH], fp32)
            out_dst = out_d[:].rearrange("(t p) d -> p t d", p=P)

            def make_epilogue(c, po_t):
                def epilogue():
                    # copy to SBUF, transpose, normalize, DMA this chunk out
                    o_sb = osb.tile([DH + 1, QC], fp32, tag="o_sb")
                    nc.vector.tensor_copy(o_sb[:], po_t[:])
                    for j in range(4):
                        pt = pt2.tile([P, DH + 1], fp32, tag="pt2")
                        nc.tensor.transpose(
                            pt[:],
                            o_sb[:, P * j : P * (j + 1)],
                            id_f32[0 : DH + 1, 0 : DH + 1],
                        )
                        recip = osb.tile([P, 1], fp32, tag="recip")
                        nc.vector.reciprocal(recip[:], pt[:, DH : DH + 1])
                        nc.vector.tensor_scalar_mul(
                            out_sb[:, 4 * c + j, :], pt[:, 0:DH], recip[:]
                        )
                    nc.sync.dma_start(
                        out_dst[:, 4 * c : 4 * (c + 1), :],
                        out_sb[:, 4 * c : 4 * (c + 1), :],
                    )

                return epilogue

            state = {"epilogue": None, "po": {}}

            def emit_span(c, r_lo, r_hi):
                # software pipeline over rounds [r_lo, r_hi]: S(r+2) is
                # issued before O(r) so the in-order PE never stalls on
                # exp(r); ACT-exp and DVE-exp rounds overlap each other.
                qs = slice(QC * c, QC * (c + 1))
                if c not in state["po"]:
                    state["po"][c] = po.tile(
                        [DH + 1, QC], fp32, tag="po", name=f"po_{c}"
                    )
                po_t = state["po"][c]
                pTs = {}

                def s_mms(r):
                    # scores for key-tile pair r -> PSUM (4-way row packing)
                    pT = pbig.tile([P, 2, QC], fp32, tag="big")
                    pTs[r] = pT
                    for i in range(2):
                        kt = 2 * r + i
                        rg = kt % 4
                        nc.tensor.matmul(
                            pT[:, i, :],
                            hT4["k"][32 * rg : 32 * (rg + 1), P * kt : P * (kt + 1)],
                            hT4["q"][32 * rg : 32 * (rg + 1), qs],
                            start=True,
                            stop=True,
                            tile_position=(32 * rg, 0),
                        )

                s_mms(r_lo)
                if r_lo + 1 <= r_hi:
                    s_mms(r_lo + 1)
                if state["epilogue"] is not None:
                    state["epilogue"]()
                    state["epilogue"] = None
                for r in range(r_lo, r_hi + 1):
                    pT = pTs.pop(r)
                    eT = expp.tile([P, 2, QC], bf16, tag="exp")
                    if r in DVE_ROUNDS:
                        # Schraudolph exp in bf16 bit-space, one DVE op
                        nc.vector.tensor_scalar(
                            eT[:].bitcast(i16),
                            pT[:],
                            EXP_A,
                            EXP_B,
                            mybir.AluOpType.mult,
                            mybir.AluOpType.add,
                        )
                    else:
                        nc.scalar.activation(
                            eT[:], pT[:], mybir.ActivationFunctionType.Exp
                        )
                    if r + 2 <= r_hi:
                        s_mms(r + 2)
                    for i in range(2):
                        kt = 2 * r + i
                        nc.tensor.matmul(
                            po_t[:],
                            vh_aug[:, kt, :],
                            eT[:, i, :],
                            start=(kt == 0),
                            stop=(kt == 2 * 8 - 1),
                        )
                if r_hi == 7:
                    state["epilogue"] = make_epilogue(c, po_t)

            setup_half(0)
            emit_span(0, 0, 3)
            setup_half(1)
            emit_span(0, 4, 7)
            for c in range(1, NQC):
                emit_span(c, 0, 7)
            state["epilogue"]()

    nc.compile()
    return nc


_NC_CACHE = None


def kernel(**inputs) -> np.ndarray:
    global _NC_CACHE
    from concourse.bass_utils import run_bass_kernel_spmd

    if _NC_CACHE is None:
        _NC_CACHE = _build()
    nc = _NC_CACHE

    in_maps = []
    for b in range(B):
        m = {
            "q": np.ascontiguousarray(inputs["q"][b], dtype=np.float32),
            "k": np.ascontiguousarray(inputs["k"][b], dtype=np.float32),
            "v": np.ascontiguousarray(inputs["v"][b], dtype=np.float32),
        }
        for w in ("Wq", "Wk", "Wv", "bq", "bk", "bv"):
            m[w] = np.ascontiguousarray(inputs[w], dtype=np.float32)
        in_maps.append(m)

    res = run_bass_kernel_spmd(nc, in_maps, core_ids=list(range(B)))
    out = np.stack([res.results[b]["out"] for b in range(B)], axis=0)
    return out


# revision 25
# speedup vs baseline: 13956.7074x; 13956.7074x over previous
"""Trainium2 Bass kernel: batch-parallel tanh-projected attention.

Reference (per batch element, 8 elements total):
    qh = tanh(q @ Wq + bq); kh = tanh(k @ Wk + bk); vh = tanh(v @ Wv + bv)
    out = softmax(qh @ kh^T, axis=-1) @ vh

Sharding: data-parallel over batch B=8 across the 8 NeuronCores; the small
256x32 projection weights are replicated.

Per-core algorithm (all in "transposed" layouts so the 2048x2048 attention
matrix never needs transposing):
  - q/k/v cast f32->bf16 during DMA (SWDGE), brought to [DIN, n] layout via
    PE transposes; the PSUM->SBUF copybacks alternate between DVE and
    ScalarE (ScalarE is idle during the input phase).
  - Projections produce hT4 = [128, 2048]: partition 32*i + c holds channel
    c of qh^T/kh^T, replicated 4x (replicated weight columns) -> enables
    4-way TensorE row-group packing for the K=32 score matmuls.
  - S^T = kh @ qh^T per key-tile pair ([128 keys, 2x512 q] PSUM); exp
    without max-subtraction (|S| <= 32 guaranteed by tanh; measured ~13).
    Half the exp tiles run on ScalarE (exact); half on DVE via a
    Schraudolph bit-trick exp in bf16 space (int16(S*a + b) bit-pattern ==
    bf16 exp approximation, one tensor_scalar op) to balance engine load.
  - The rounds are software-pipelined (scores for round r+2 issue before
    the output matmuls of round r) so the in-order TensorE never stalls on
    exp, and ACT-exp and DVE-exp rounds overlap.
  - O^T accumulated as [vh | 1]^T @ exp(S^T): the ones column yields the
    softmax denominator for free.
  - PE-transpose O^T chunks, divide by denominator, per-chunk output DMA.
  - Setup is progressive: first halves of k/q/v load+project, then chunk 0
    rounds 0-3 are emitted, then the second setup half, so the main loop
    overlaps the input phase.

Measured (8 cores, axon/PJRT): relative error 1.40e-2 vs the fp32
reference. Cost-model (TimelineSim) predicted single-core duration ~62 us;
engine busy: PE 40us (model serializes the row-group-packed score matmuls
that real HW runs concurrently), ACT 31.5us, DVE 30.8us.
"""

import numpy as np

B, N, M, DIN, DH = 8, 2048, 2048, 256, 32
P = 128
NT = N // P  # 16 row tiles
QC = 512  # q-chunk (matmul moving dim)
NQC = N // QC  # 4

# Schraudolph bf16-space exp: bitcast(int16(x * 128*log2(e) + (127*128 - C)))
EXP_A = float(128.0 / np.log(2.0))
EXP_B = float(127.0 * 128.0 - 5.25)
DVE_ROUNDS = (1, 3, 5, 7)  # rounds (of 8 per q-chunk) whose exp runs on DVE


def _build():
    import concourse.mybir as mybir
    import concourse.tile as tile
    from concourse import bacc
    from concourse.masks import make_identity

    fp32 = mybir.dt.float32
    bf16 = mybir.dt.bfloat16
    i16 = mybir.dt.int16

    nc = bacc.Bacc("TRN2", target_bir_lowering=False, debug=False)

    q_d = nc.dram_tensor("q", [N, DIN], fp32, kind="ExternalInput")
    k_d = nc.dram_tensor("k", [M, DIN], fp32, kind="ExternalInput")
    v_d = nc.dram_tensor("v", [M, DIN], fp32, kind="ExternalInput")
    wq_d = nc.dram_tensor("Wq", [DIN, DH], fp32, kind="ExternalInput")
    wk_d = nc.dram_tensor("Wk", [DIN, DH], fp32, kind="ExternalInput")
    wv_d = nc.dram_tensor("Wv", [DIN, DH], fp32, kind="ExternalInput")
    bq_d = nc.dram_tensor("bq", [DH], fp32, kind="ExternalInput")
    bk_d = nc.dram_tensor("bk", [DH], fp32, kind="ExternalInput")
    bv_d = nc.dram_tensor("bv", [DH], fp32, kind="ExternalInput")
    out_d = nc.dram_tensor("out", [N, DH], fp32, kind="ExternalOutput")

    xdram = {"q": q_d, "k": k_d, "v": v_d}
    wdram = {"q": wq_d, "k": wk_d, "v": wv_d}
    bdram = {"q": bq_d, "k": bk_d, "v": bv_d}

    with tile.TileContext(nc) as tc:
        with (
            tc.tile_pool(name="const", bufs=1) as const,
            tc.tile_pool(name="stage", bufs=1) as stage,
            tc.tile_pool(name="sb", bufs=1) as sb,
            tc.tile_pool(name="expp", bufs=4) as expp,
            tc.tile_pool(name="osb", bufs=2) as osb,
            tc.tile_pool(name="pbig", bufs=3, space="PSUM") as pbig,
            tc.tile_pool(name="po", bufs=1, space="PSUM") as po,
            tc.tile_pool(name="pt2", bufs=1, space="PSUM") as pt2,
        ):
            # ---- constants ----
            id_bf = const.tile([P, P], bf16)
            make_identity(nc, id_bf[:])
            id_f32 = const.tile([P, P], fp32)
            make_identity(nc, id_f32[:])

            w4 = {}
            bias = {}
            for name in ("q", "k", "v"):
                wf = const.tile([P, 2, DH], fp32, tag=f"wf_{name}", name=f"wf_{name}")
                nc.sync.dma_start(
                    wf[:], wdram[name][:].rearrange("(o p) c -> p o c", p=P)
                )
                w4t = const.tile(
                    [P, 2, 4 * DH], bf16, tag=f"w4_{name}", name=f"w4_{name}"
                )
                for j in range(4):
                    nc.vector.tensor_copy(w4t[:, :, j * DH : (j + 1) * DH], wf[:])
                w4[name] = w4t

                bt = const.tile([P, 1], fp32, tag=f"b_{name}", name=f"b_{name}")
                for i in range(4):
                    nc.sync.dma_start(
                        bt[i * DH : (i + 1) * DH, :],
                        bdram[name][:].rearrange("(c one) -> c one", one=1),
                    )
                bias[name] = bt

            xT = {}
            hT4 = {}
            for name in ("q", "k", "v"):
                xT[name] = sb.tile([P, 2, N], bf16, tag=f"xT_{name}", name=f"xT_{name}")
                hT4[name] = sb.tile([P, N], bf16, tag=f"hT4_{name}", name=f"hT4_{name}")

            # Input path: SWDGE cast-DMA chunks (f32->bf16), PE-transpose
            # 128x128 tiles, copy PSUM->SBUF alternating DVE/ScalarE (splits
            # the copyback load across both engines; ScalarE is idle during
            # the input phase since exp hasn't started).
            def load_and_transpose(name, g):
                src = xdram[name][:].rearrange("(t p) d -> p t d", p=P)
                xbf = stage.tile(
                    [P, 4, DIN], bf16, tag=f"xbf_{name}_{g}",
                    name=f"xbf_{name}_{g}",
                )
                nc.gpsimd.dma_start(xbf[:], src[:, 4 * g : 4 * g + 4, :])
                for o in range(2):
                    ptp = pbig.tile([P, 4, P], bf16, tag="big")
                    for i in range(4):
                        nc.tensor.transpose(
                            ptp[:, i, :],
                            xbf[:, i, o * P : (o + 1) * P],
                            id_bf[:],
                        )
                    dst = xT[name][:, o, 512 * g : 512 * (g + 1)]
                    if (g + o) % 2 == 0:
                        nc.vector.tensor_copy(dst, ptp[:])
                    else:
                        nc.scalar.copy(dst, ptp[:])

            def project(name, ch):
                # hT4 = tanh(W4^T @ xT + b), bf16, 4x-replicated channels
                ph = pbig.tile([P, 2, QC], fp32, tag="big")
                for nh in range(2):
                    for o in range(2):
                        nc.tensor.matmul(
                            ph[:, nh, :],
                            w4[name][:, o, :],
                            xT[name][
                                :, o, 1024 * ch + 512 * nh : 1024 * ch + 512 * (nh + 1)
                            ],
                            start=(o == 0),
                            stop=(o == 1),
                        )
                nc.scalar.activation(
                    hT4[name][:, 1024 * ch : 1024 * (ch + 1)].rearrange(
                        "p (a b) -> p a b", a=2
                    ),
                    ph[:],
                    mybir.ActivationFunctionType.Tanh,
                    bias=bias[name][:],
                )

            # vh_aug: [P, NT, DH+1] bf16 (row-major vh tiles + ones col)
            vh_aug = sb.tile([P, NT, DH + 1], bf16)
            nc.gpsimd.memset(vh_aug[:, :, DH : DH + 1], 1.0)

            def vh_aug_fill(g):
                pv = pbig.tile([P, 4, DH], bf16, tag="big")
                for i in range(4):
                    kt = 4 * g + i
                    nc.tensor.transpose(
                        pv[:, i, :],
                        hT4["v"][0:DH, P * kt : P * (kt + 1)],
                        id_bf[0:DH, 0:DH],
                    )
                nc.vector.tensor_copy(vh_aug[:, 4 * g : 4 * g + 4, 0:DH], pv[:])

            # Progressive setup: the main loop's round r of any chunk needs
            # only key/value tiles 2r,2r+1 and q-chunk c — so stage the
            # first halves of k/q/v (casts g0,g1 -> proj ch0 -> vh_aug
            # tiles 0-7), then EMIT chunk 0's rounds 0-3 before the second
            # setup half, so main-loop work (and its PSUM slot requests)
            # interleaves with the remaining input processing.
            def setup_half(h):
                for g in (2 * h, 2 * h + 1):
                    for name in ("k", "q", "v"):
                        load_and_transpose(name, g)
                for name in ("k", "q", "v"):
                    project(name, h)
                vh_aug_fill(2 * h)
                vh_aug_fill(2 * h + 1)

            # ---- main attention loop ----
            out_sb = sb.tile([P, NT, DH], fp32)
            out_dst = out_d[:].rearrange("(t p) d -> p t d", p=P)

            def make_epilogue(c, po_t):
                def epilogue():
                    # copy to SBUF, transpose, normalize, DMA this chunk out
                    o_sb = osb.tile([DH + 1, QC], fp32, tag="o_sb")
                    nc.vector.tensor_copy(o_sb[:], po_t[:])
                    for j in range(4):
                        pt = pt2.tile([P, DH + 1], fp32, tag="pt2")
                        nc.tensor.transpose(
                            pt[:],
                            o_sb[:, P * j : P * (j + 1)],
                            id_f32[0 : DH + 1, 0 : DH + 1],
                        )
                        recip = osb.tile([P, 1], fp32, tag="recip")
                        nc.vector.reciprocal(recip[:], pt[:, DH : DH + 1])
                        nc.vector.tensor_scalar_mul(
                            out_sb[:, 4 * c + j, :], pt[:, 0:DH], recip[:]
                        )
                    nc.sync.dma_start(
                        out_dst[:, 4 * c : 4 * (c + 1), :],
                        out_sb[:, 4 * c : 4 * (c + 1), :],
                    )

                return epilogue

            state = {"epilogue": None, "po": {}}

            def emit_span(c, r_lo, r_hi):
                # software pipeline over rounds [r_lo, r_hi]: S(r+2) is
                # issued before O(r) so the in-order PE never stalls on
                # exp(r); ACT-exp and DVE-exp rounds overlap each other.
                qs = slice(QC * c, QC * (c + 1))
                if c not in state["po"]:
                    state["po"][c] = po.tile(
                        [DH + 1, QC], fp32, tag="po", name=f"po_{c}"
                    )
                po_t = state["po"][c]
                pTs = {}

                def s_mms(r):
                    # scores for key-tile pair r -> PSUM (4-way row packing)
                    pT = pbig.tile([P, 2, QC], fp32, tag="big")
                    pTs[r] = pT
                    for i in range(2):
                        kt = 2 * r + i
                        rg = kt % 4
                        nc.tensor.matmul(
                            pT[:, i, :],
                            hT4["k"][32 * rg : 32 * (rg + 1), P * kt : P * (kt + 1)],
                            hT4["q"][32 * rg : 32 * (rg + 1), qs],
                            start=True,
                            stop=True,
                            tile_position=(32 * rg, 0),
                        )

                s_mms(r_lo)
                if r_lo + 1 <= r_hi:
                    s_mms(r_lo + 1)
                if state["epilogue"] is not None:
                    state["epilogue"]()
                    state["epilogue"] = None
                for r in range(r_lo, r_hi + 1):
                    pT = pTs.pop(r)
                    eT = expp.tile([P, 2, QC], bf16, tag="exp")
                    if r in DVE_ROUNDS:
                        # Schraudolph exp in bf16 bit-space, one DVE op
                        nc.vector.tensor_scalar(
                            eT[:].bitcast(i16),
                            pT[:],
                            EXP_A,
                            EXP_B,
                            mybir.AluOpType.mult,
                            mybir.AluOpType.add,
                        )
                    else:
                        nc.scalar.activation(
                            eT[:], pT[:], mybir.ActivationFunctionType.Exp
                        )
                    if r + 2 <= r_hi:
                        s_mms(r + 2)
                    for i in range(2):
                        kt = 2 * r + i
                        nc.tensor.matmul(
                            po_t[:],
                            vh_aug[:, kt, :],
                            eT[:, i, :],
                            start=(kt == 0),
                            stop=(kt == 2 * 8 - 1),
                        )
                if r_hi == 7:
                    state["epilogue"] = make_epilogue(c, po_t)

            setup_half(0)
            emit_span(0, 0, 3)
            setup_half(1)
            emit_span(0, 4, 7)
            for c in range(1, NQC):
                emit_span(c, 0, 7)
            state["epilogue"]()

    nc.compile()
    return nc


_NC_CACHE = None


def kernel(**inputs) -> np.ndarray:
    global _NC_CACHE
    from concourse.bass_utils import run_bass_kernel_spmd

    if _NC_CACHE is None:
        _NC_CACHE = _build()
    nc = _NC_CACHE

    in_maps = []
    for b in range(B):
        m = {
            "q": np.ascontiguousarray(inputs["q"][b], dtype=np.float32),
            "k": np.ascontiguousarray(inputs["k"][b], dtype=np.float32),
            "v": np.ascontiguousarray(inputs["v"][b], dtype=np.float32),
        }
        for w in ("Wq", "Wk", "Wv", "bq", "bk", "bv"):
            m[w] = np.ascontiguousarray(inputs[w], dtype=np.float32)
        in_maps.append(m)

    res = run_bass_kernel_spmd(nc, in_maps, core_ids=list(range(B)))
    out = np.stack([res.results[b]["out"] for b in range(B)], axis=0)
    return out


# revision 27
# speedup vs baseline: 14142.4276x; 1.0133x over previous
"""Trainium2 Bass kernel: batch-parallel tanh-projected attention.

Reference (per batch element, 8 elements total):
    qh = tanh(q @ Wq + bq); kh = tanh(k @ Wk + bk); vh = tanh(v @ Wv + bv)
    out = softmax(qh @ kh^T, axis=-1) @ vh

Sharding: data-parallel over batch B=8 across the 8 NeuronCores; the small
256x32 projection weights are replicated.

Per-core algorithm (all in "transposed" layouts so the 2048x2048 attention
matrix never needs transposing):
  - q/k/v cast f32->bf16 during DMA (SWDGE), brought to [DIN, n] layout via
    PE transposes; the PSUM->SBUF copybacks alternate between DVE and
    ScalarE (ScalarE is idle during the input phase).
  - Projections produce hT4 = [128, 2048]: partition 32*i + c holds channel
    c of qh^T/kh^T, replicated 4x (replicated weight columns) -> enables
    4-way TensorE row-group packing for the K=32 score matmuls.
  - S^T = kh @ qh^T per key-tile pair ([128 keys, 2x512 q] PSUM); exp
    without max-subtraction (|S| <= 32 guaranteed by tanh; measured ~13).
    Half the exp tiles run on ScalarE (exact); half on DVE via a
    Schraudolph bit-trick exp in bf16 space (int16(S*a + b) bit-pattern ==
    bf16 exp approximation, one tensor_scalar op) to balance engine load.
  - The rounds are software-pipelined (scores for round r+2 issue before
    the output matmuls of round r) so the in-order TensorE never stalls on
    exp, and ACT-exp and DVE-exp rounds overlap.
  - O^T accumulated as [vh | 1]^T @ exp(S^T): the ones column yields the
    softmax denominator for free.
  - PE-transpose O^T chunks, divide by denominator, per-chunk output DMA.
  - Setup is progressive: first halves of k/q/v load+project, then chunk 0
    rounds 0-3 are emitted, then the second setup half, so the main loop
    overlaps the input phase.

Measured (8 cores, axon/PJRT): relative error 1.40e-2 vs the fp32
reference. Cost-model (TimelineSim) predicted single-core duration ~62 us;
engine busy: PE 40us (model serializes the row-group-packed score matmuls
that real HW runs concurrently), ACT 31.5us, DVE 30.8us.
"""

import numpy as np

B, N, M, DIN, DH = 8, 2048, 2048, 256, 32
P = 128
NT = N // P  # 16 row tiles
QC = 512  # q-chunk (matmul moving dim)
NQC = N // QC  # 4

# Schraudolph bf16-space exp: bitcast(int16(x * 128*log2(e) + (127*128 - C)))
EXP_A = float(128.0 / np.log(2.0))
EXP_B = float(127.0 * 128.0 - 5.25)
DVE_ROUNDS = (1, 3, 5, 7)  # rounds (of 8 per q-chunk) whose exp runs on DVE


def _build():
    import concourse.mybir as mybir
    import concourse.tile as tile
    from concourse import bacc
    from concourse.masks import make_identity

    fp32 = mybir.dt.float32
    bf16 = mybir.dt.bfloat16
    i16 = mybir.dt.int16

    nc = bacc.Bacc("TRN2", target_bir_lowering=False, debug=False)

    q_d = nc.dram_tensor("q", [N, DIN], fp32, kind="ExternalInput")
    k_d = nc.dram_tensor("k", [M, DIN], fp32, kind="ExternalInput")
    v_d = nc.dram_tensor("v", [M, DIN], fp32, kind="ExternalInput")
    wq_d = nc.dram_tensor("Wq", [DIN, DH], fp32, kind="ExternalInput")
    wk_d = nc.dram_tensor("Wk", [DIN, DH], fp32, kind="ExternalInput")
    wv_d = nc.dram_tensor("Wv", [DIN, DH], fp32, kind="ExternalInput")
    bq_d = nc.dram_tensor("bq", [DH], fp32, kind="ExternalInput")
    bk_d = nc.dram_tensor("bk", [DH], fp32, kind="ExternalInput")
    bv_d = nc.dram_tensor("bv", [DH], fp32, kind="ExternalInput")
    out_d = nc.dram_tensor("out", [N, DH], fp32, kind="ExternalOutput")

    xdram = {"q": q_d, "k": k_d, "v": v_d}
    wdram = {"q": wq_d, "k": wk_d, "v": wv_d}
    bdram = {"q": bq_d, "k": bk_d, "v": bv_d}

    with tile.TileContext(nc) as tc:
        with (
            tc.tile_pool(name="const", bufs=1) as const,
            tc.tile_pool(name="stage", bufs=1) as stage,
            tc.tile_pool(name="sb", bufs=1) as sb,
            tc.tile_pool(name="expp", bufs=4) as expp,
            tc.tile_pool(name="osb", bufs=2) as osb,
            tc.tile_pool(name="pbig", bufs=3, space="PSUM") as pbig,
            tc.tile_pool(name="po", bufs=1, space="PSUM") as po,
            tc.tile_pool(name="pt2", bufs=1, space="PSUM") as pt2,
        ):
            # ---- constants ----
            id_bf = const.tile([P, P], bf16)
            make_identity(nc, id_bf[:])
            id_f32 = const.tile([P, P], fp32)

            w4 = {}
            bias = {}
            for name in ("q", "k", "v"):
                wf = const.tile([P, 2, DH], fp32, tag=f"wf_{name}", name=f"wf_{name}")
                nc.sync.dma_start(
                    wf[:], wdram[name][:].rearrange("(o p) c -> p o c", p=P)
                )
                w4t = const.tile(
                    [P, 2, 4 * DH], bf16, tag=f"w4_{name}", name=f"w4_{name}"
                )
                for j in range(4):
                    nc.vector.tensor_copy(w4t[:, :, j * DH : (j + 1) * DH], wf[:])
                w4[name] = w4t

                bt = const.tile([P, 1], fp32, tag=f"b_{name}", name=f"b_{name}")
                for i in range(4):
                    nc.sync.dma_start(
                        bt[i * DH : (i + 1) * DH, :],
                        bdram[name][:].rearrange("(c one) -> c one", one=1),
                    )
                bias[name] = bt

            xT = {}
            hT4 = {}
            for name in ("q", "k", "v"):
                xT[name] = sb.tile([P, 2, N], bf16, tag=f"xT_{name}", name=f"xT_{name}")
                hT4[name] = sb.tile([P, N], bf16, tag=f"hT4_{name}", name=f"hT4_{name}")

            # Input path: SWDGE cast-DMA chunks (f32->bf16), PE-transpose
            # 128x128 tiles, copy PSUM->SBUF alternating DVE/ScalarE (splits
            # the copyback load across both engines; ScalarE is idle during
            # the input phase since exp hasn't started).
            def load_and_transpose(name, g):
                src = xdram[name][:].rearrange("(t p) d -> p t d", p=P)
                xbf = stage.tile(
                    [P, 4, DIN], bf16, tag=f"xbf_{name}_{g}",
                    name=f"xbf_{name}_{g}",
                )
                nc.gpsimd.dma_start(xbf[:], src[:, 4 * g : 4 * g + 4, :])
                for o in range(2):
                    ptp = pbig.tile([P, 4, P], bf16, tag="big")
                    for i in range(4):
                        nc.tensor.transpose(
                            ptp[:, i, :],
                            xbf[:, i, o * P : (o + 1) * P],
                            id_bf[:],
                        )
                    dst = xT[name][:, o, 512 * g : 512 * (g + 1)]
                    if (g + o) % 2 == 0:
                        nc.vector.tensor_copy(dst, ptp[:])
                    else:
                        nc.scalar.copy(dst, ptp[:])

            def project(name, ch):
                # hT4 = tanh(W4^T @ xT + b), bf16, 4x-replicated channels
                ph = pbig.tile([P, 2, QC], fp32, tag="big")
                for nh in range(2):
                    for o in range(2):
                        nc.tensor.matmul(
                            ph[:, nh, :],
                            w4[name][:, o, :],
                            xT[name][
                                :, o, 1024 * ch + 512 * nh : 1024 * ch + 512 * (nh + 1)
                            ],
                            start=(o == 0),
                            stop=(o == 1),
                        )
                nc.scalar.activation(
                    hT4[name][:, 1024 * ch : 1024 * (ch + 1)].rearrange(
                        "p (a b) -> p a b", a=2
                    ),
                    ph[:],
                    mybir.ActivationFunctionType.Tanh,
                    bias=bias[name][:],
                )

            # vh_aug: [P, NT, DH+1] bf16 (row-major vh tiles + ones col)
            vh_aug = sb.tile([P, NT, DH + 1], bf16)
            _vh_ones = {"done": False}

            def vh_aug_fill(g):
                if not _vh_ones["done"]:
                    nc.gpsimd.memset(vh_aug[:, :, DH : DH + 1], 1.0)
                    _vh_ones["done"] = True
                pv = pbig.tile([P, 4, DH], bf16, tag="big")
                for i in range(4):
                    kt = 4 * g + i
                    nc.tensor.transpose(
                        pv[:, i, :],
                        hT4["v"][0:DH, P * kt : P * (kt + 1)],
                        id_bf[0:DH, 0:DH],
                    )
                nc.vector.tensor_copy(vh_aug[:, 4 * g : 4 * g + 4, 0:DH], pv[:])

            # Progressive setup: the main loop's round r of any chunk needs
            # only key/value tiles 2r,2r+1 and q-chunk c — so stage the
            # first halves of k/q/v (casts g0,g1 -> proj ch0 -> vh_aug
            # tiles 0-7), then EMIT chunk 0's rounds 0-3 before the second
            # setup half, so main-loop work (and its PSUM slot requests)
            # interleaves with the remaining input processing.
            def setup_half(h):
                for g in (2 * h, 2 * h + 1):
                    for name in ("k", "q", "v"):
                        load_and_transpose(name, g)
                for name in ("k", "q", "v"):
                    project(name, h)
                vh_aug_fill(2 * h)
                vh_aug_fill(2 * h + 1)

            # ---- main attention loop ----
            out_sb = sb.tile([P, NT, DH], fp32)
            out_dst = out_d[:].rearrange("(t p) d -> p t d", p=P)

            def make_epilogue(c, po_t):
                def epilogue():
                    # copy to SBUF, transpose, normalize, DMA this chunk out
                    o_sb = osb.tile([DH + 1, QC], fp32, tag="o_sb")
                    nc.vector.tensor_copy(o_sb[:], po_t[:])
                    for j in range(4):
                        pt = pt2.tile([P, DH + 1], fp32, tag="pt2")
                        nc.tensor.transpose(
                            pt[:],
                            o_sb[:, P * j : P * (j + 1)],
                            id_f32[0 : DH + 1, 0 : DH + 1],
                        )
                        recip = osb.tile([P, 1], fp32, tag="recip")
                        nc.vector.reciprocal(recip[:], pt[:, DH : DH + 1])
                        nc.vector.tensor_scalar_mul(
                            out_sb[:, 4 * c + j, :], pt[:, 0:DH], recip[:]
                        )
                    nc.sync.dma_start(
                        out_dst[:, 4 * c : 4 * (c + 1), :],
                        out_sb[:, 4 * c : 4 * (c + 1), :],
                    )

                return epilogue

            state = {"epilogue": None, "po": {}}

            def emit_span(c, r_lo, r_hi):
                # software pipeline over rounds [r_lo, r_hi]: S(r+2) is
                # issued before O(r) so the in-order PE never stalls on
                # exp(r); ACT-exp and DVE-exp rounds overlap each other.
                qs = slice(QC * c, QC * (c + 1))
                if c not in state["po"]:
                    state["po"][c] = po.tile(
                        [DH + 1, QC], fp32, tag="po", name=f"po_{c}"
                    )
                po_t = state["po"][c]
                pTs = {}

                def s_mms(r):
                    # scores for key-tile pair r -> PSUM (4-way row packing)
                    pT = pbig.tile([P, 2, QC], fp32, tag="big")
                    pTs[r] = pT
                    for i in range(2):
                        kt = 2 * r + i
                        rg = kt % 4
                        nc.tensor.matmul(
                            pT[:, i, :],
                            hT4["k"][32 * rg : 32 * (rg + 1), P * kt : P * (kt + 1)],
                            hT4["q"][32 * rg : 32 * (rg + 1), qs],
                            start=True,
                            stop=True,
                            tile_position=(32 * rg, 0),
                        )

                s_mms(r_lo)
                if r_lo + 1 <= r_hi:
                    s_mms(r_lo + 1)
                if state["epilogue"] is not None:
                    state["epilogue"]()
                    state["epilogue"] = None
                for r in range(r_lo, r_hi + 1):
                    pT = pTs.pop(r)
                    eT = expp.tile([P, 2, QC], bf16, tag="exp")
                    if r in DVE_ROUNDS:
                        # Schraudolph exp in bf16 bit-space, one DVE op
                        nc.vector.tensor_scalar(
                            eT[:].bitcast(i16),
                            pT[:],
                            EXP_A,
                            EXP_B,
                            mybir.AluOpType.mult,
                            mybir.AluOpType.add,
                        )
                    else:
                        nc.scalar.activation(
                            eT[:], pT[:], mybir.ActivationFunctionType.Exp
                        )
                    if r + 2 <= r_hi:
                        s_mms(r + 2)
                    for i in range(2):
                        kt = 2 * r + i
                        nc.tensor.matmul(
                            po_t[:],
                            vh_aug[:, kt, :],
                            eT[:, i, :],
                            start=(kt == 0),
                            stop=(kt == 2 * 8 - 1),
                        )
                if r_hi == 7:
                    state["epilogue"] = make_epilogue(c, po_t)

            setup_half(0)
            make_identity(nc, id_f32[:])
            emit_span(0, 0, 3)
            setup_half(1)
            emit_span(0, 4, 7)
            for c in range(1, NQC):
                emit_span(c, 0, 7)
            state["epilogue"]()

    nc.compile()
    return nc


_NC_CACHE = None


def kernel(**inputs) -> np.ndarray:
    global _NC_CACHE
    from concourse.bass_utils import run_bass_kernel_spmd

    if _NC_CACHE is None:
        _NC_CACHE = _build()
    nc = _NC_CACHE

    in_maps = []
    for b in range(B):
        m = {
            "q": np.ascontiguousarray(inputs["q"][b], dtype=np.float32),
            "k": np.ascontiguousarray(inputs["k"][b], dtype=np.float32),
            "v": np.ascontiguousarray(inputs["v"][b], dtype=np.float32),
        }
        for w in ("Wq", "Wk", "Wv", "bq", "bk", "bv"):
            m[w] = np.ascontiguousarray(inputs[w], dtype=np.float32)
        in_maps.append(m)

    res = run_bass_kernel_spmd(nc, in_maps, core_ids=list(range(B)))
    out = np.stack([res.results[b]["out"] for b in range(B)], axis=0)
    return out


# revision 28
# speedup vs baseline: 14210.8557x; 1.0048x over previous
"""Trainium2 Bass kernel: batch-parallel tanh-projected attention.

Reference (per batch element, 8 elements total):
    qh = tanh(q @ Wq + bq); kh = tanh(k @ Wk + bk); vh = tanh(v @ Wv + bv)
    out = softmax(qh @ kh^T, axis=-1) @ vh

Sharding: data-parallel over batch B=8 across the 8 NeuronCores; the small
256x32 projection weights are replicated.

Per-core algorithm (all in "transposed" layouts so the 2048x2048 attention
matrix never needs transposing):
  - q/k/v cast f32->bf16 during DMA (SWDGE), brought to [DIN, n] layout via
    PE transposes; the PSUM->SBUF copybacks alternate between DVE and
    ScalarE (ScalarE is idle during the input phase).
  - Projections produce hT4 = [128, 2048]: partition 32*i + c holds channel
    c of qh^T/kh^T, replicated 4x (replicated weight columns) -> enables
    4-way TensorE row-group packing for the K=32 score matmuls.
  - S^T = kh @ qh^T per key-tile pair ([128 keys, 2x512 q] PSUM); exp
    without max-subtraction (|S| <= 32 guaranteed by tanh; measured ~13).
    Half the exp tiles run on ScalarE (exact); half on DVE via a
    Schraudolph bit-trick exp in bf16 space (int16(S*a + b) bit-pattern ==
    bf16 exp approximation, one tensor_scalar op) to balance engine load.
  - The rounds are software-pipelined (scores for round r+2 issue before
    the output matmuls of round r) so the in-order TensorE never stalls on
    exp, and ACT-exp and DVE-exp rounds overlap.
  - O^T accumulated as [vh | 1]^T @ exp(S^T): the ones column yields the
    softmax denominator for free.
  - PE-transpose O^T chunks, divide by denominator, per-chunk output DMA.
  - Setup is progressive: first halves of k/q/v load+project, then chunk 0
    rounds 0-3 are emitted, then the second setup half, so the main loop
    overlaps the input phase.

Measured (8 cores, axon/PJRT): relative error 1.40e-2 vs the fp32
reference. Cost-model (TimelineSim) predicted single-core duration ~62 us;
engine busy: PE 40us (model serializes the row-group-packed score matmuls
that real HW runs concurrently), ACT 31.5us, DVE 30.8us.
"""

import numpy as np

B, N, M, DIN, DH = 8, 2048, 2048, 256, 32
P = 128
NT = N // P  # 16 row tiles
QC = 512  # q-chunk (matmul moving dim)
NQC = N // QC  # 4

# Schraudolph bf16-space exp: bitcast(int16(x * 128*log2(e) + (127*128 - C)))
EXP_A = float(128.0 / np.log(2.0))
EXP_B = float(127.0 * 128.0 - 5.25)
# rounds (of 8 per q-chunk) whose exp runs on DVE, per chunk parity
DVE_ROUNDS_BY_CHUNK = {0: (1, 3, 5, 7), 1: (1, 4, 7), 2: (1, 3, 5, 7), 3: (1, 4, 7)}


def _build():
    import concourse.mybir as mybir
    import concourse.tile as tile
    from concourse import bacc
    from concourse.masks import make_identity

    fp32 = mybir.dt.float32
    bf16 = mybir.dt.bfloat16
    i16 = mybir.dt.int16

    nc = bacc.Bacc("TRN2", target_bir_lowering=False, debug=False)

    q_d = nc.dram_tensor("q", [N, DIN], fp32, kind="ExternalInput")
    k_d = nc.dram_tensor("k", [M, DIN], fp32, kind="ExternalInput")
    v_d = nc.dram_tensor("v", [M, DIN], fp32, kind="ExternalInput")
    wq_d = nc.dram_tensor("Wq", [DIN, DH], fp32, kind="ExternalInput")
    wk_d = nc.dram_tensor("Wk", [DIN, DH], fp32, kind="ExternalInput")
    wv_d = nc.dram_tensor("Wv", [DIN, DH], fp32, kind="ExternalInput")
    bq_d = nc.dram_tensor("bq", [DH], fp32, kind="ExternalInput")
    bk_d = nc.dram_tensor("bk", [DH], fp32, kind="ExternalInput")
    bv_d = nc.dram_tensor("bv", [DH], fp32, kind="ExternalInput")
    out_d = nc.dram_tensor("out", [N, DH], fp32, kind="ExternalOutput")

    xdram = {"q": q_d, "k": k_d, "v": v_d}
    wdram = {"q": wq_d, "k": wk_d, "v": wv_d}
    bdram = {"q": bq_d, "k": bk_d, "v": bv_d}

    with tile.TileContext(nc) as tc:
        with (
            tc.tile_pool(name="const", bufs=1) as const,
            tc.tile_pool(name="stage", bufs=1) as stage,
            tc.tile_pool(name="sb", bufs=1) as sb,
            tc.tile_pool(name="expp", bufs=6) as expp,
            tc.tile_pool(name="osb", bufs=2) as osb,
            tc.tile_pool(name="pbig", bufs=3, space="PSUM") as pbig,
            tc.tile_pool(name="po", bufs=1, space="PSUM") as po,
            tc.tile_pool(name="pt2", bufs=1, space="PSUM") as pt2,
        ):
            # ---- constants ----
            id_bf = const.tile([P, P], bf16)
            make_identity(nc, id_bf[:])
            id_f32 = const.tile([P, P], fp32)

            w4 = {}
            bias = {}
            for name in ("q", "k", "v"):
                wf = const.tile([P, 2, DH], fp32, tag=f"wf_{name}", name=f"wf_{name}")
                nc.sync.dma_start(
                    wf[:], wdram[name][:].rearrange("(o p) c -> p o c", p=P)
                )
                w4t = const.tile(
                    [P, 2, 4 * DH], bf16, tag=f"w4_{name}", name=f"w4_{name}"
                )
                for j in range(4):
                    nc.vector.tensor_copy(w4t[:, :, j * DH : (j + 1) * DH], wf[:])
                w4[name] = w4t

                bt = const.tile([P, 1], fp32, tag=f"b_{name}", name=f"b_{name}")
                for i in range(4):
                    nc.sync.dma_start(
                        bt[i * DH : (i + 1) * DH, :],
                        bdram[name][:].rearrange("(c one) -> c one", one=1),
                    )
                bias[name] = bt

            xT = {}
            hT4 = {}
            for name in ("q", "k", "v"):
                xT[name] = sb.tile([P, 2, N], bf16, tag=f"xT_{name}", name=f"xT_{name}")
                hT4[name] = sb.tile([P, N], bf16, tag=f"hT4_{name}", name=f"hT4_{name}")

            # Input path: SWDGE cast-DMA chunks (f32->bf16), PE-transpose
            # 128x128 tiles, copy PSUM->SBUF alternating DVE/ScalarE (splits
            # the copyback load across both engines; ScalarE is idle during
            # the input phase since exp hasn't started).
            def load_and_transpose(name, g):
                src = xdram[name][:].rearrange("(t p) d -> p t d", p=P)
                xbf = stage.tile(
                    [P, 4, DIN], bf16, tag=f"xbf_{name}_{g}",
                    name=f"xbf_{name}_{g}",
                )
                nc.gpsimd.dma_start(xbf[:], src[:, 4 * g : 4 * g + 4, :])
                for o in range(2):
                    ptp = pbig.tile([P, 4, P], bf16, tag="big")
                    for i in range(4):
                        nc.tensor.transpose(
                            ptp[:, i, :],
                            xbf[:, i, o * P : (o + 1) * P],
                            id_bf[:],
                        )
                    dst = xT[name][:, o, 512 * g : 512 * (g + 1)]
                    if (2 * g + o) % 3 == 2:
                        nc.scalar.copy(dst, ptp[:])
                    else:
                        nc.vector.tensor_copy(dst, ptp[:])

            def project(name, ch):
                # hT4 = tanh(W4^T @ xT + b), bf16, 4x-replicated channels
                ph = pbig.tile([P, 2, QC], fp32, tag="big")
                for nh in range(2):
                    for o in range(2):
                        nc.tensor.matmul(
                            ph[:, nh, :],
                            w4[name][:, o, :],
                            xT[name][
                                :, o, 1024 * ch + 512 * nh : 1024 * ch + 512 * (nh + 1)
                            ],
                            start=(o == 0),
                            stop=(o == 1),
                        )
                nc.scalar.activation(
                    hT4[name][:, 1024 * ch : 1024 * (ch + 1)].rearrange(
                        "p (a b) -> p a b", a=2
                    ),
                    ph[:],
                    mybir.ActivationFunctionType.Tanh,
                    bias=bias[name][:],
                )

            # vh_aug: [P, NT, DH+1] bf16 (row-major vh tiles + ones col)
            vh_aug = sb.tile([P, NT, DH + 1], bf16)
            _vh_ones = {"done": False}

            def vh_aug_fill(g):
                if not _vh_ones["done"]:
                    nc.gpsimd.memset(vh_aug[:, :, DH : DH + 1], 1.0)
                    _vh_ones["done"] = True
                pv = pbig.tile([P, 4, DH], bf16, tag="big")
                for i in range(4):
                    kt = 4 * g + i
                    nc.tensor.transpose(
                        pv[:, i, :],
                        hT4["v"][0:DH, P * kt : P * (kt + 1)],
                        id_bf[0:DH, 0:DH],
                    )
                nc.vector.tensor_copy(vh_aug[:, 4 * g : 4 * g + 4, 0:DH], pv[:])

            # Progressive setup: the main loop's round r of any chunk needs
            # only key/value tiles 2r,2r+1 and q-chunk c — so stage the
            # first halves of k/q/v (casts g0,g1 -> proj ch0 -> vh_aug
            # tiles 0-7), then EMIT chunk 0's rounds 0-3 before the second
            # setup half, so main-loop work (and its PSUM slot requests)
            # interleaves with the remaining input processing.
            def setup_half(h):
                for g in (2 * h, 2 * h + 1):
                    for name in ("k", "q", "v"):
                        load_and_transpose(name, g)
                for name in ("k", "q", "v"):
                    project(name, h)
                vh_aug_fill(2 * h)
                vh_aug_fill(2 * h + 1)

            # ---- main attention loop ----
            out_sb = sb.tile([P, NT, DH], fp32)
            out_dst = out_d[:].rearrange("(t p) d -> p t d", p=P)

            def make_epilogue(c, po_t):
                def epilogue():
                    # copy to SBUF, transpose, normalize, DMA this chunk out
                    o_sb = osb.tile([DH + 1, QC], fp32, tag="o_sb")
                    nc.vector.tensor_copy(o_sb[:], po_t[:])
                    for j in range(4):
                        pt = pt2.tile([P, DH + 1], fp32, tag="pt2")
                        nc.tensor.transpose(
                            pt[:],
                            o_sb[:, P * j : P * (j + 1)],
                            id_f32[0 : DH + 1, 0 : DH + 1],
                        )
                        recip = osb.tile([P, 1], fp32, tag="recip")
                        nc.vector.reciprocal(recip[:], pt[:, DH : DH + 1])
                        nc.vector.tensor_scalar_mul(
                            out_sb[:, 4 * c + j, :], pt[:, 0:DH], recip[:]
                        )
                    nc.sync.dma_start(
                        out_dst[:, 4 * c : 4 * (c + 1), :],
                        out_sb[:, 4 * c : 4 * (c + 1), :],
                    )

                return epilogue

            state = {"epilogue": None, "po": {}}

            def emit_span(c, r_lo, r_hi):
                # software pipeline over rounds [r_lo, r_hi]: S(r+2) is
                # issued before O(r) so the in-order PE never stalls on
                # exp(r); ACT-exp and DVE-exp rounds overlap each other.
                qs = slice(QC * c, QC * (c + 1))
                if c not in state["po"]:
                    state["po"][c] = po.tile(
                        [DH + 1, QC], fp32, tag="po", name=f"po_{c}"
                    )
                po_t = state["po"][c]
                pTs = {}

                def s_mms(r):
                    # scores for key-tile pair r -> PSUM (4-way row packing)
                    pT = pbig.tile([P, 2, QC], fp32, tag="big")
                    pTs[r] = pT
                    for i in range(2):
                        kt = 2 * r + i
                        rg = kt % 4
                        nc.tensor.matmul(
                            pT[:, i, :],
                            hT4["k"][32 * rg : 32 * (rg + 1), P * kt : P * (kt + 1)],
                            hT4["q"][32 * rg : 32 * (rg + 1), qs],
                            start=True,
                            stop=True,
                            tile_position=(32 * rg, 0),
                        )

                s_mms(r_lo)
                if r_lo + 1 <= r_hi:
                    s_mms(r_lo + 1)
                if state["epilogue"] is not None:
                    state["epilogue"]()
                    state["epilogue"] = None
                for r in range(r_lo, r_hi + 1):
                    pT = pTs.pop(r)
                    eT = expp.tile([P, 2, QC], bf16, tag="exp")
                    if r in DVE_ROUNDS_BY_CHUNK[c]:
                        # Schraudolph exp in bf16 bit-space, one DVE op
                        nc.vector.tensor_scalar(
                            eT[:].bitcast(i16),
                            pT[:],
                            EXP_A,
                            EXP_B,
                            mybir.AluOpType.mult,
                            mybir.AluOpType.add,
                        )
                    else:
                        nc.scalar.activation(
                            eT[:], pT[:], mybir.ActivationFunctionType.Exp
                        )
                    if r + 2 <= r_hi:
                        s_mms(r + 2)
                    for i in range(2):
                        kt = 2 * r + i
                        nc.tensor.matmul(
                            po_t[:],
                            vh_aug[:, kt, :],
                            eT[:, i, :],
                            start=(kt == 0),
                            stop=(kt == 2 * 8 - 1),
                        )
                if r_hi == 7:
                    state["epilogue"] = make_epilogue(c, po_t)

            setup_half(0)
            make_identity(nc, id_f32[:])
            emit_span(0, 0, 3)
            setup_half(1)
            emit_span(0, 4, 7)
            for c in range(1, NQC):
                emit_span(c, 0, 7)
            state["epilogue"]()

    nc.compile()
    return nc


_NC_CACHE = None


def kernel(**inputs) -> np.ndarray:
    global _NC_CACHE
    from concourse.bass_utils import run_bass_kernel_spmd

    if _NC_CACHE is None:
        _NC_CACHE = _build()
    nc = _NC_CACHE

    in_maps = []
    for b in range(B):
        m = {
            "q": np.ascontiguousarray(inputs["q"][b], dtype=np.float32),
            "k": np.ascontiguousarray(inputs["k"][b], dtype=np.float32),
            "v": np.ascontiguousarray(inputs["v"][b], dtype=np.float32),
        }
        for w in ("Wq", "Wk", "Wv", "bq", "bk", "bv"):
            m[w] = np.ascontiguousarray(inputs[w], dtype=np.float32)
        in_maps.append(m)

    res = run_bass_kernel_spmd(nc, in_maps, core_ids=list(range(B)))
    out = np.stack([res.results[b]["out"] for b in range(B)], axis=0)
    return out


# revision 29
# speedup vs baseline: 14317.8133x; 1.0075x over previous
"""Trainium2 Bass kernel: batch-parallel tanh-projected attention.

Reference (per batch element, 8 elements total):
    qh = tanh(q @ Wq + bq); kh = tanh(k @ Wk + bk); vh = tanh(v @ Wv + bv)
    out = softmax(qh @ kh^T, axis=-1) @ vh

Sharding: data-parallel over batch B=8 across the 8 NeuronCores; the small
256x32 projection weights are replicated.

Per-core algorithm (all in "transposed" layouts so the 2048x2048 attention
matrix never needs transposing):
  - q/k/v cast f32->bf16 during DMA (SWDGE), brought to [DIN, n] layout via
    PE transposes; the PSUM->SBUF copybacks alternate between DVE and
    ScalarE (ScalarE is idle during the input phase).
  - Projections produce hT4 = [128, 2048]: partition 32*i + c holds channel
    c of qh^T/kh^T, replicated 4x (replicated weight columns) -> enables
    4-way TensorE row-group packing for the K=32 score matmuls.
  - S^T = kh @ qh^T per key-tile pair ([128 keys, 2x512 q] PSUM); exp
    without max-subtraction (|S| <= 32 guaranteed by tanh; measured ~13).
    Half the exp tiles run on ScalarE (exact); half on DVE via a
    Schraudolph bit-trick exp in bf16 space (int16(S*a + b) bit-pattern ==
    bf16 exp approximation, one tensor_scalar op) to balance engine load.
  - The rounds are software-pipelined (scores for round r+2 issue before
    the output matmuls of round r) so the in-order TensorE never stalls on
    exp, and ACT-exp and DVE-exp rounds overlap.
  - O^T accumulated as [vh | 1]^T @ exp(S^T): the ones column yields the
    softmax denominator for free.
  - PE-transpose O^T chunks, divide by denominator, per-chunk output DMA.
  - Setup is progressive: first halves of k/q/v load+project, then chunk 0
    rounds 0-3 are emitted, then the second setup half, so the main loop
    overlaps the input phase.

Measured (8 cores, axon/PJRT): relative error 1.40e-2 vs the fp32
reference. Cost-model (TimelineSim) predicted single-core duration ~62 us;
engine busy: PE 40us (model serializes the row-group-packed score matmuls
that real HW runs concurrently), ACT 31.5us, DVE 30.8us.
"""

import numpy as np

B, N, M, DIN, DH = 8, 2048, 2048, 256, 32
P = 128
NT = N // P  # 16 row tiles
QC = 512  # q-chunk (matmul moving dim)
NQC = N // QC  # 4

# Schraudolph bf16-space exp: bitcast(int16(x * 128*log2(e) + (127*128 - C)))
EXP_A = float(128.0 / np.log(2.0))
EXP_B = float(127.0 * 128.0 - 5.25)
# rounds (of 8 per q-chunk) whose exp runs on DVE, per chunk parity
DVE_ROUNDS_BY_CHUNK = {0: (1, 3, 5, 7), 1: (1, 4, 7), 2: (1, 3, 5, 7), 3: (1, 4, 7)}


def _build():
    import concourse.mybir as mybir
    import concourse.tile as tile
    from concourse import bacc
    from concourse.masks import make_identity

    fp32 = mybir.dt.float32
    bf16 = mybir.dt.bfloat16
    i16 = mybir.dt.int16

    nc = bacc.Bacc("TRN2", target_bir_lowering=False, debug=False)

    q_d = nc.dram_tensor("q", [N, DIN], fp32, kind="ExternalInput")
    k_d = nc.dram_tensor("k", [M, DIN], fp32, kind="ExternalInput")
    v_d = nc.dram_tensor("v", [M, DIN], fp32, kind="ExternalInput")
    wq_d = nc.dram_tensor("Wq", [DIN, DH], fp32, kind="ExternalInput")
    wk_d = nc.dram_tensor("Wk", [DIN, DH], fp32, kind="ExternalInput")
    wv_d = nc.dram_tensor("Wv", [DIN, DH], fp32, kind="ExternalInput")
    bq_d = nc.dram_tensor("bq", [DH], fp32, kind="ExternalInput")
    bk_d = nc.dram_tensor("bk", [DH], fp32, kind="ExternalInput")
    bv_d = nc.dram_tensor("bv", [DH], fp32, kind="ExternalInput")
    out_d = nc.dram_tensor("out", [N, DH], fp32, kind="ExternalOutput")

    xdram = {"q": q_d, "k": k_d, "v": v_d}
    wdram = {"q": wq_d, "k": wk_d, "v": wv_d}
    bdram = {"q": bq_d, "k": bk_d, "v": bv_d}

    with tile.TileContext(nc) as tc:
        with (
            tc.tile_pool(name="const", bufs=1) as const,
            tc.tile_pool(name="stage", bufs=1) as stage,
            tc.tile_pool(name="sb", bufs=1) as sb,
            tc.tile_pool(name="expp", bufs=6) as expp,
            tc.tile_pool(name="osb", bufs=2) as osb,
            tc.tile_pool(name="pbig", bufs=3, space="PSUM") as pbig,
            tc.tile_pool(name="po", bufs=1, space="PSUM") as po,
            tc.tile_pool(name="pt2", bufs=1, space="PSUM") as pt2,
        ):
            # ---- constants ----
            id_bf = const.tile([P, P], bf16)
            make_identity(nc, id_bf[:])
            id_f32 = const.tile([P, P], fp32)

            w4 = {}
            bias = {}
            for name in ("q", "k", "v"):
                wf = const.tile([P, 2, DH], fp32, tag=f"wf_{name}", name=f"wf_{name}")
                nc.sync.dma_start(
                    wf[:], wdram[name][:].rearrange("(o p) c -> p o c", p=P)
                )
                w4t = const.tile(
                    [P, 2, 4 * DH], bf16, tag=f"w4_{name}", name=f"w4_{name}"
                )
                for j in range(4):
                    nc.vector.tensor_copy(w4t[:, :, j * DH : (j + 1) * DH], wf[:])
                w4[name] = w4t

                bt = const.tile([P, 1], fp32, tag=f"b_{name}", name=f"b_{name}")
                for i in range(4):
                    nc.sync.dma_start(
                        bt[i * DH : (i + 1) * DH, :],
                        bdram[name][:].rearrange("(c one) -> c one", one=1),
                    )
                bias[name] = bt

            xT = {}
            hT4 = {}
            for name in ("q", "k", "v"):
                xT[name] = sb.tile([P, 2, N], bf16, tag=f"xT_{name}", name=f"xT_{name}")
                hT4[name] = sb.tile([P, N], bf16, tag=f"hT4_{name}", name=f"hT4_{name}")

            # Input path: SWDGE cast-DMA chunks (f32->bf16), PE-transpose
            # 128x128 tiles, copy PSUM->SBUF alternating DVE/ScalarE (splits
            # the copyback load across both engines; ScalarE is idle during
            # the input phase since exp hasn't started).
            def load_and_transpose(name, g):
                src = xdram[name][:].rearrange("(t p) d -> p t d", p=P)
                xbf = stage.tile(
                    [P, 4, DIN], bf16, tag=f"xbf_{name}_{g}",
                    name=f"xbf_{name}_{g}",
                )
                nc.gpsimd.dma_start(xbf[:], src[:, 4 * g : 4 * g + 4, :])
                for o in range(2):
                    ptp = pbig.tile([P, 4, P], bf16, tag="big")
                    for i in range(4):
                        nc.tensor.transpose(
                            ptp[:, i, :],
                            xbf[:, i, o * P : (o + 1) * P],
                            id_bf[:],
                        )
                    dst = xT[name][:, o, 512 * g : 512 * (g + 1)]
                    if (2 * g + o) % 3 == 2:
                        nc.scalar.copy(dst, ptp[:])
                    else:
                        nc.vector.tensor_copy(dst, ptp[:])

            def project(name, ch):
                # hT4 = tanh(W4^T @ xT + b), bf16, 4x-replicated channels
                ph = pbig.tile([P, 2, QC], fp32, tag="big")
                for nh in range(2):
                    for o in range(2):
                        nc.tensor.matmul(
                            ph[:, nh, :],
                            w4[name][:, o, :],
                            xT[name][
                                :, o, 1024 * ch + 512 * nh : 1024 * ch + 512 * (nh + 1)
                            ],
                            start=(o == 0),
                            stop=(o == 1),
                        )
                nc.scalar.activation(
                    hT4[name][:, 1024 * ch : 1024 * (ch + 1)].rearrange(
                        "p (a b) -> p a b", a=2
                    ),
                    ph[:],
                    mybir.ActivationFunctionType.Tanh,
                    bias=bias[name][:],
                )

            # vh_aug: [P, NT, DH+1] bf16 (row-major vh tiles + ones col)
            vh_aug = sb.tile([P, NT, DH + 1], bf16)
            _vh_ones = {"done": False}

            def vh_aug_fill(g):
                if not _vh_ones["done"]:
                    nc.gpsimd.memset(vh_aug[:, :, DH : DH + 1], 1.0)
                    _vh_ones["done"] = True
                pv = pbig.tile([P, 4, DH], bf16, tag="big")
                for i in range(4):
                    kt = 4 * g + i
                    nc.tensor.transpose(
                        pv[:, i, :],
                        hT4["v"][0:DH, P * kt : P * (kt + 1)],
                        id_bf[0:DH, 0:DH],
                    )
                nc.vector.tensor_copy(vh_aug[:, 4 * g : 4 * g + 4, 0:DH], pv[:])

            # Progressive setup: the main loop's round r of any chunk needs
            # only key/value tiles 2r,2r+1 and q-chunk c — so stage the
            # first halves of k/q/v (casts g0,g1 -> proj ch0 -> vh_aug
            # tiles 0-7), then EMIT chunk 0's rounds 0-3 before the second
            # setup half, so main-loop work (and its PSUM slot requests)
            # interleaves with the remaining input processing.
            def setup_half(h):
                # k and q gate the score matmuls: put both their cast groups
                # ahead of v in the serial SWDGE queue and project them
                # first, so chunk-0 exps can start while v still loads.
                for name in ("k", "q"):
                    for g in (2 * h, 2 * h + 1):
                        load_and_transpose(name, g)
                project("k", h)
                project("q", h)
                for g in (2 * h, 2 * h + 1):
                    load_and_transpose("v", g)
                project("v", h)
                vh_aug_fill(2 * h)
                vh_aug_fill(2 * h + 1)

            # ---- main attention loop ----
            out_sb = sb.tile([P, NT, DH], fp32)
            out_dst = out_d[:].rearrange("(t p) d -> p t d", p=P)

            def make_epilogue(c, po_t):
                def epilogue():
                    # copy to SBUF, transpose, normalize, DMA this chunk out
                    o_sb = osb.tile([DH + 1, QC], fp32, tag="o_sb")
                    nc.vector.tensor_copy(o_sb[:], po_t[:])
                    for j in range(4):
                        pt = pt2.tile([P, DH + 1], fp32, tag="pt2")
                        nc.tensor.transpose(
                            pt[:],
                            o_sb[:, P * j : P * (j + 1)],
                            id_f32[0 : DH + 1, 0 : DH + 1],
                        )
                        recip = osb.tile([P, 1], fp32, tag="recip")
                        nc.vector.reciprocal(recip[:], pt[:, DH : DH + 1])
                        nc.vector.tensor_scalar_mul(
                            out_sb[:, 4 * c + j, :], pt[:, 0:DH], recip[:]
                        )
                    nc.sync.dma_start(
                        out_dst[:, 4 * c : 4 * (c + 1), :],
                        out_sb[:, 4 * c : 4 * (c + 1), :],
                    )

                return epilogue

            state = {"epilogue": None, "po": {}}

            def emit_span(c, r_lo, r_hi):
                # software pipeline over rounds [r_lo, r_hi]: S(r+2) is
                # issued before O(r) so the in-order PE never stalls on
                # exp(r); ACT-exp and DVE-exp rounds overlap each other.
                qs = slice(QC * c, QC * (c + 1))
                if c not in state["po"]:
                    state["po"][c] = po.tile(
                        [DH + 1, QC], fp32, tag="po", name=f"po_{c}"
                    )
                po_t = state["po"][c]
                pTs = {}

                def s_mms(r):
                    # scores for key-tile pair r -> PSUM (4-way row packing)
                    pT = pbig.tile([P, 2, QC], fp32, tag="big")
                    pTs[r] = pT
                    for i in range(2):
                        kt = 2 * r + i
                        rg = kt % 4
                        nc.tensor.matmul(
                            pT[:, i, :],
                            hT4["k"][32 * rg : 32 * (rg + 1), P * kt : P * (kt + 1)],
                            hT4["q"][32 * rg : 32 * (rg + 1), qs],
                            start=True,
                            stop=True,
                            tile_position=(32 * rg, 0),
                        )

                s_mms(r_lo)
                if r_lo + 1 <= r_hi:
                    s_mms(r_lo + 1)
                if state["epilogue"] is not None:
                    state["epilogue"]()
                    state["epilogue"] = None
                for r in range(r_lo, r_hi + 1):
                    pT = pTs.pop(r)
                    eT = expp.tile([P, 2, QC], bf16, tag="exp")
                    if r in DVE_ROUNDS_BY_CHUNK[c]:
                        # Schraudolph exp in bf16 bit-space, one DVE op
                        nc.vector.tensor_scalar(
                            eT[:].bitcast(i16),
                            pT[:],
                            EXP_A,
                            EXP_B,
                            mybir.AluOpType.mult,
                            mybir.AluOpType.add,
                        )
                    else:
                        nc.scalar.activation(
                            eT[:], pT[:], mybir.ActivationFunctionType.Exp
                        )
                    if r + 2 <= r_hi:
                        s_mms(r + 2)
                    for i in range(2):
                        kt = 2 * r + i
                        nc.tensor.matmul(
                            po_t[:],
                            vh_aug[:, kt, :],
                            eT[:, i, :],
                            start=(kt == 0),
                            stop=(kt == 2 * 8 - 1),
                        )
                if r_hi == 7:
                    state["epilogue"] = make_epilogue(c, po_t)

            setup_half(0)
            make_identity(nc, id_f32[:])
            emit_span(0, 0, 3)
            setup_half(1)
            emit_span(0, 4, 7)
            for c in range(1, NQC):
                emit_span(c, 0, 7)
            state["epilogue"]()

    nc.compile()
    return nc


_NC_CACHE = None


def kernel(**inputs) -> np.ndarray:
    global _NC_CACHE
    from concourse.bass_utils import run_bass_kernel_spmd

    if _NC_CACHE is None:
        _NC_CACHE = _build()
    nc = _NC_CACHE

    in_maps = []
    for b in range(B):
        m = {
            "q": np.ascontiguousarray(inputs["q"][b], dtype=np.float32),
            "k": np.ascontiguousarray(inputs["k"][b], dtype=np.float32),
            "v": np.ascontiguousarray(inputs["v"][b], dtype=np.float32),
        }
        for w in ("Wq", "Wk", "Wv", "bq", "bk", "bv"):
            m[w] = np.ascontiguousarray(inputs[w], dtype=np.float32)
        in_maps.append(m)

    res = run_bass_kernel_spmd(nc, in_maps, core_ids=list(range(B)))
    out = np.stack([res.results[b]["out"] for b in range(B)], axis=0)
    return out
